# revision 1
# baseline (speedup 1.0000x reference)
"""GNN message-passing kernel for 8 Trainium2 NeuronCores (Bass/Tile).

Takes FULL inputs, shards nodes across 8 cores internally, runs the
4-layer GNN (dense -> spmm -> spmm -> dense) with two bf16 AllGathers
of the hidden node table, and PE-matmul-based weighted segment sums
(host-built one-hot selector matrices), then gathers the full output.
"""

import math
from contextlib import ExitStack
from dataclasses import dataclass

import ml_dtypes
import numpy as np

import concourse.bass as bass
import concourse.mybir as mybir
import concourse.tile as tile
from concourse import bacc
from concourse.bass_utils import run_bass_kernel_spmd
from concourse.masks import make_identity

BF16 = ml_dtypes.bfloat16
AF = mybir.ActivationFunctionType


@dataclass(frozen=True)
class Cfg:
    n_nodes: int = 50000
    n_edges: int = 800000
    in_dim: int = 512
    h1: int = 512
    h2: int = 256
    out_dim: int = 128
    n_cores: int = 8
    group_blocks: int = 3  # row-blocks per gather group

    @property
    def nodes_per_core(self):
        return math.ceil(self.n_nodes / self.n_cores)

    @property
    def npad(self):  # per-core padded nodes
        return math.ceil(self.nodes_per_core / 128) * 128

    @property
    def nblocks(self):
        return self.npad // 128

    @property
    def ntot(self):
        return self.npad * self.n_cores

    @property
    def half(self):
        return self.ntot // 2

    @property
    def ngroups(self):
        return math.ceil(self.nblocks / self.group_blocks)


FULL = Cfg()


# ---------------------------------------------------------------- host prep


def edge_structure(cfg: Cfg, edge_row, edge_col, edge_weight):
    """Per-core edge streams with SPMD-uniform chunk counts.

    Returns (meta, per_core) where meta has the uniform chunk structure:
      meta['nch'][g][h]      total chunks in gather call (group g, half h)
      meta['chunk_blk'][g][h] list of block ids (one per chunk, ordered)
      meta['off16'][g][h]    idx-tile column offset (int16 cols) of the call
      meta['totch']          total chunks
      meta['idxcols']        total int16 columns of the idx tensor
    per_core[c] = dict(idx=[128, idxcols] int16, pmat=[128, totch*128] bf16)
    """
    nc_, npad, half, nb, G = (
        cfg.n_cores,
        cfg.npad,
        cfg.half,
        cfg.nblocks,
        cfg.group_blocks,
    )
    npc = cfg.nodes_per_core
    assert half <= 32767, "half-table must fit int16 indices"

    core_of = edge_row // npc
    lr_all = edge_row - core_of * npc  # local row
    cp_all = (edge_col // npc) * npad + (edge_col % npc)  # padded global col
    half_all = (cp_all >= half).astype(np.int64)
    cl_all = cp_all - half_all * half  # local col within half-table

    # bucket edges per (core, block, half)
    per = {}
    counts = np.zeros((nc_, nb, 2), np.int64)
    for c in range(nc_):
        m = core_of == c
        lr, cl, hf, w = lr_all[m], cl_all[m], half_all[m], edge_weight[m]
        blk = lr // 128
        order = np.lexsort((lr, hf, blk))
        per[c] = (lr[order], cl[order], hf[order], w[order], blk[order])
        np.add.at(counts[c], (blk, hf), 1)

    # uniform chunks per (block, half) = max over cores
    chunks_bh = np.ceil(counts / 128.0).astype(np.int64).max(axis=0)  # [nb, 2]

    ngroups = cfg.ngroups
    nch = [[0, 0] for _ in range(ngroups)]
    chunk_blk = [[[], []] for _ in range(ngroups)]
    off16 = [[0, 0] for _ in range(ngroups)]
    tot16 = 0
    totch = 0
    for g in range(ngroups):
        blocks = range(g * G, min((g + 1) * G, nb))
        for h in (0, 1):
            off16[g][h] = tot16
            n = 0
            for b in blocks:
                cb = int(chunks_bh[b, h])
                chunk_blk[g][h].extend([b] * cb)
                n += cb
            nch[g][h] = n
            tot16 += n * 8  # 128 idx per chunk -> 8 int16 cols
            totch += n

    meta = dict(
        nch=nch,
        chunk_blk=chunk_blk,
        off16=off16,
        totch=totch,
        idxcols=max(tot16, 8),
        chunks_bh=chunks_bh,
    )

    per_core = []
    for c in range(nc_):
        lr, cl, hf, w, blk = per[c]
        idx_flat = np.zeros(meta["idxcols"] * 16, np.int16)
        pmat = np.zeros((128, totch * 128), BF16)
        # walk (group, half, block) in kernel order
        jchunk = 0
        for g in range(ngroups):
            blocks = range(g * G, min((g + 1) * G, nb))
            for h in (0, 1):
                call_vals = []
                for b in blocks:
                    sel = (blk == b) & (hf == h)
                    e_cl = cl[sel]
                    e_lr = lr[sel]
                    e_w = w[sel]
                    cb = int(chunks_bh[b, h])
                    ne = len(e_cl)
                    pad = cb * 128 - ne
                    vals = np.concatenate([e_cl, np.zeros(pad, np.int64)])
                    call_vals.append(vals)
                    # P matrices: edge slot i of chunk j -> col (local row)
                    if ne > 0:
                        i_in = np.arange(ne)
                        jj = jchunk + i_in // 128
                        slot = i_in % 128
                        r = e_lr - b * 128
                        pmat[slot, jj * 128 + r] = e_w.astype(BF16)
                    jchunk += cb
                if not call_vals:
                    continue
                v = np.concatenate(call_vals)
                n = len(v)
                if n == 0:
                    continue
                base16 = meta["off16"][g][h]
                i_in = np.arange(n)
                idx_flat[(base16 + i_in // 16) * 16 + (i_in % 16)] = v.astype(
                    np.int16
                )
        idx_mat = idx_flat.reshape(meta["idxcols"], 16).T  # [16, idxcols]
        idx_mat = np.tile(idx_mat, (8, 1))  # replicate to 128 partitions
        per_core.append(dict(idx=np.ascontiguousarray(idx_mat), pmat=pmat))

    return meta, per_core


def prep_inputs(cfg: Cfg, inputs):
    """Shard + lay out all per-core input tensors."""
    f = inputs["features"].astype(np.float32)
    meta, per_edge = edge_structure(
        cfg,
        inputs["edge_row"].astype(np.int64),
        inputs["edge_col"].astype(np.int64),
        inputs["edge_weight"].astype(np.float32),
    )
    kin = cfg.in_dim // 128
    k1 = cfg.h1 // 128
    k2 = cfg.h2 // 128

    def wlayout(w, kt):
        # [K, M] -> [128, kt*M] with [:, i*M:(i+1)*M] = w[i*128:(i+1)*128]
        K, M = w.shape
        return (
            w.reshape(kt, 128, M).transpose(1, 0, 2).reshape(128, kt * M)
        ).astype(BF16)

    w1 = wlayout(inputs["W_lin1"].astype(np.float32), kin)
    wg1 = wlayout(inputs["W_g1"].astype(np.float32), k1)
    wg2 = wlayout(inputs["W_g2"].astype(np.float32), k2)
    wl2 = wlayout(inputs["W_lin2"].astype(np.float32), k2)
    b1 = (
        inputs["b_lin1"].astype(np.float32).reshape(kin, 128).T.copy()
    )  # [128, kin]
    bg1 = inputs["b_g1"].astype(BF16).reshape(1, cfg.h2)
    bg2 = inputs["b_g2"].astype(BF16).reshape(1, cfg.h2)
    bl2 = inputs["b_lin2"].astype(BF16).reshape(1, cfg.out_dim)

    npc, npad = cfg.nodes_per_core, cfg.npad
    in_maps = []
    for c in range(cfg.n_cores):
        lo = c * npc
        hi = min((c + 1) * npc, cfg.n_nodes)
        xc = np.zeros((npad, cfg.in_dim), np.float32)
        xc[: hi - lo] = f[lo:hi]
        # XT layout [128, kin*npad]: [:, i*npad:(i+1)*npad] = x.T[i*128:...]
        xt = (
            xc.T.reshape(kin, 128, npad)
            .transpose(1, 0, 2)
            .reshape(128, kin * npad)
        ).astype(BF16)
        in_maps.append(
            {
                "xt": np.ascontiguousarray(xt),
                "w1": w1,
                "wg1": wg1,
                "wg2": wg2,
                "wl2": wl2,
                "b1": b1,
                "bg1": bg1,
                "bg2": bg2,
                "bl2": bl2,
                "idx": per_edge[c]["idx"],
                "pmat": per_edge[c]["pmat"],
            }
        )
    return meta, in_maps


# ---------------------------------------------------------------- kernel IR


def build(cfg: Cfg, meta):
    nc = bacc.Bacc(
        "TRN2",
        target_bir_lowering=False,
        debug=False,
        num_devices=cfg.n_cores,
    )
    bf = mybir.dt.bfloat16
    f32 = mybir.dt.float32
    i16 = mybir.dt.int16
    kin = cfg.in_dim // 128
    k1 = cfg.h1 // 128
    k2 = cfg.h2 // 128
    npad, nb, G, H2, OUT = (
        cfg.npad,
        cfg.nblocks,
        cfg.group_blocks,
        cfg.h2,
        cfg.out_dim,
    )
    HALF = cfg.half
    totch = meta["totch"]

    xt_d = nc.dram_tensor("xt", [128, kin * npad], bf, kind="ExternalInput").ap()
    w1_d = nc.dram_tensor("w1", [128, kin * cfg.h1], bf, kind="ExternalInput").ap()
    wg1_d = nc.dram_tensor("wg1", [128, k1 * H2], bf, kind="ExternalInput").ap()
    wg2_d = nc.dram_tensor("wg2", [128, k2 * H2], bf, kind="ExternalInput").ap()
    wl2_d = nc.dram_tensor("wl2", [128, k2 * OUT], bf, kind="ExternalInput").ap()
    b1_d = nc.dram_tensor("b1", [128, kin], f32, kind="ExternalInput").ap()
    bg1_d = nc.dram_tensor("bg1", [1, H2], bf, kind="ExternalInput").ap()
    bg2_d = nc.dram_tensor("bg2", [1, H2], bf, kind="ExternalInput").ap()
    bl2_d = nc.dram_tensor("bl2", [1, OUT], bf, kind="ExternalInput").ap()
    idx_d = nc.dram_tensor(
        "idx", [128, meta["idxcols"]], i16, kind="ExternalInput"
    ).ap()
    pmat_d = nc.dram_tensor(
        "pmat", [128, totch * 128], bf, kind="ExternalInput"
    ).ap()
    y_d = nc.dram_tensor("y", [npad, OUT], f32, kind="ExternalOutput").ap()

    g1_local = nc.dram_tensor("g1_local", [npad, H2], bf).ap()
    g2_local = nc.dram_tensor("g2_local", [npad, H2], bf).ap()
    g1_table = nc.dram_tensor(
        "g1_table", [cfg.ntot, H2], bf, addr_space="Shared"
    ).ap()
    g2_table = nc.dram_tensor(
        "g2_table", [cfg.ntot, H2], bf, addr_space="Shared"
    ).ap()

    rg = [list(range(cfg.n_cores))]

    def spmm(tc, ctx, nc, table, idx_s, ones_t, brow, psum_tag, out_cb):
        """Weighted segment-sum of gathered table rows, per row-block."""
        gp = [
            ctx.enter_context(tc.tile_pool(name=f"gath{psum_tag}{h}", bufs=2))
            for h in (0, 1)
        ]
        pp = ctx.enter_context(tc.tile_pool(name=f"pm{psum_tag}", bufs=2))
        sp = ctx.enter_context(
            tc.tile_pool(name=f"ps{psum_tag}", bufs=G + 1, space="PSUM")
        )
        j0 = 0
        for g in range(cfg.ngroups):
            blocks = list(range(g * G, min((g + 1) * G, nb)))
            gt = {}
            for h in (0, 1):
                n = meta["nch"][g][h]
                if n == 0:
                    continue
                t = gp[h].tile([128, n, H2], bf, tag="g")
                # split into <=15-chunk (1920-idx) sub-calls: a single
                # dma_gather must fit the SWDGE descriptor ring
                for lo in range(0, n, 15):
                    ns = min(15, n - lo)
                    o16 = meta["off16"][g][h] + lo * 8
                    nc.gpsimd.dma_gather(
                        out_ap=t[:, lo : lo + ns, :],
                        in_ap=table[h * HALF : (h + 1) * HALF, :],
                        idxs_ap=idx_s[:, o16 : o16 + ns * 8],
                        num_idxs=ns * 128,
                        num_idxs_reg=ns * 128,
                        elem_size=H2,
                        single_packet=False,
                    )
                gt[h] = t
            chg = meta["nch"][g][0] + meta["nch"][g][1]
            if chg > 0:
                ptile = pp.tile([128, chg * 128], bf, tag="p")
                nc.sync.dma_start(
                    ptile[:], pmat_d[:, j0 * 128 : (j0 + chg) * 128]
                )
            psums = {
                b: sp.tile([128, H2], f32, tag="ps", name=f"ps{psum_tag}_{b}")
                for b in blocks
            }
            started = dict.fromkeys(blocks, False)
            jj = 0
            for h in (0, 1):
                for jh, b in enumerate(meta["chunk_blk"][g][h]):
                    nc.tensor.matmul(
                        psums[b][:],
                        lhsT=ptile[:, jj * 128 : (jj + 1) * 128],
                        rhs=gt[h][:, jh, :],
                        start=not started[b],
                        stop=False,
                    )
                    started[b] = True
                    jj += 1
            for b in blocks:
                nc.tensor.matmul(
                    psums[b][:],
                    lhsT=ones_t[:1, :],
                    rhs=brow[:1, :],
                    start=not started[b],
                    stop=True,
                )
                out_cb(b, psums[b])
            j0 += chg

    with tile.TileContext(nc) as tc:
        with ExitStack() as top:
            const = top.enter_context(tc.tile_pool(name="const", bufs=1))
            w1_s = const.tile([128, kin * cfg.h1], bf)
            nc.sync.dma_start(w1_s[:], w1_d[:, :])
            wg1_s = const.tile([128, k1 * H2], bf)
            nc.sync.dma_start(wg1_s[:], wg1_d[:, :])
            wg2_s = const.tile([128, k2 * H2], bf)
            nc.sync.dma_start(wg2_s[:], wg2_d[:, :])
            wl2_s = const.tile([128, k2 * OUT], bf)
            nc.sync.dma_start(wl2_s[:], wl2_d[:, :])
            b1_s = const.tile([128, kin], f32)
            nc.sync.dma_start(b1_s[:], b1_d[:, :])
            bg1_s = const.tile([1, H2], bf)
            nc.sync.dma_start(bg1_s[:], bg1_d[:, :])
            bg2_s = const.tile([1, H2], bf)
            nc.sync.dma_start(bg2_s[:], bg2_d[:, :])
            bl2_s = const.tile([1, OUT], bf)
            nc.sync.dma_start(bl2_s[:], bl2_d[:, :])
            idx_s = const.tile([128, meta["idxcols"]], i16)
            nc.sync.dma_start(idx_s[:], idx_d[:, :])
            ident = const.tile([128, 128], bf)
            make_identity(nc, ident[:])
            ones_t = const.tile([1, 128], bf)
            nc.gpsimd.memset(ones_t[:], 1.0)

            # ---------------- L1: h1T[f, n] = sigmoid(W1.T @ X.T + b1)
            with ExitStack() as ph1:
                h1p = ph1.enter_context(tc.tile_pool(name="h1t", bufs=1))
                h1t = h1p.tile([128, k1 * npad], bf)
                with ExitStack() as px:
                    xp = px.enter_context(tc.tile_pool(name="xt", bufs=1))
                    psp = px.enter_context(
                        tc.tile_pool(name="ps1", bufs=4, space="PSUM")
                    )
                    xt_s = xp.tile([128, kin * npad], bf)
                    nc.sync.dma_start(xt_s[:], xt_d[:, :])
                    nsl = [(i * 512, min((i + 1) * 512, npad)) for i in range(math.ceil(npad / 512))]
                    for f1t in range(k1):
                        for a, b_ in nsl:
                            nw = b_ - a
                            ps = psp.tile([128, 512], f32, tag="ps")
                            for kt in range(kin):
                                nc.tensor.matmul(
                                    ps[:, :nw],
                                    lhsT=w1_s[
                                        :,
                                        kt * cfg.h1
                                        + f1t * 128 : kt * cfg.h1
                                        + f1t * 128
                                        + 128,
                                    ],
                                    rhs=xt_s[:, kt * npad + a : kt * npad + b_],
                                    start=(kt == 0),
                                    stop=(kt == kin - 1),
                                )
                            nc.scalar.activation(
                                h1t[:, f1t * npad + a : f1t * npad + b_],
                                ps[:, :nw],
                                AF.Sigmoid,
                                bias=b1_s[:, f1t : f1t + 1],
                            )

                # ---------------- L2a: g1[n, h2] = h1 @ Wg1  (lhsT = h1T)
                with ExitStack() as p2:
                    psp2 = p2.enter_context(
                        tc.tile_pool(name="ps2", bufs=4, space="PSUM")
                    )
                    tp2 = p2.enter_context(tc.tile_pool(name="g1t", bufs=3))
                    for b in range(nb):
                        ps = psp2.tile([128, H2], f32, tag="ps")
                        for kt in range(k1):
                            nc.tensor.matmul(
                                ps[:],
                                lhsT=h1t[
                                    :, kt * npad + b * 128 : kt * npad + b * 128 + 128
                                ],
                                rhs=wg1_s[:, kt * H2 : (kt + 1) * H2],
                                start=(kt == 0),
                                stop=(kt == k1 - 1),
                            )
                        g1tile = tp2.tile([128, H2], bf, tag="g1")
                        nc.vector.tensor_copy(g1tile[:], ps[:])
                        nc.sync.dma_start(
                            g1_local[b * 128 : (b + 1) * 128, :], g1tile[:]
                        )

            nc.gpsimd.collective_compute(
                "AllGather",
                mybir.AluOpType.bypass,
                replica_groups=rg,
                ins=[g1_local[:, :]],
                outs=[g1_table[:, :]],
            )

            # ---------------- spmm1 -> h2 (resident)
            with ExitStack() as ph2:
                h2p = ph2.enter_context(tc.tile_pool(name="h2res", bufs=1))
                h2r = h2p.tile([128, nb * H2], bf)

                with ExitStack() as ps1:
                    def cb1(b, psum):
                        nc.scalar.activation(
                            h2r[:, b * H2 : (b + 1) * H2], psum[:], AF.Relu
                        )

                    spmm(tc, ps1, nc, g1_table, idx_s, ones_t, bg1_s, "a", cb1)

                # ------------- L3a: g2 = h2 @ Wg2 (transpose h2 per block)
                with ExitStack() as p3:
                    tps = p3.enter_context(
                        tc.tile_pool(name="tps", bufs=2, space="PSUM")
                    )
                    psp3 = p3.enter_context(
                        tc.tile_pool(name="ps3", bufs=2, space="PSUM")
                    )
                    tp3 = p3.enter_context(tc.tile_pool(name="l3t", bufs=3))
                    for b in range(nb):
                        h2T = tp3.tile([128, k2, 128], bf, tag="h2T")
                        for kt in range(k2):
                            pt = tps.tile([128, 128], bf, tag="pt")
                            nc.tensor.transpose(
                                pt[:],
                                h2r[:, b * H2 + kt * 128 : b * H2 + (kt + 1) * 128],
                                ident[:],
                            )
                            nc.vector.tensor_copy(h2T[:, kt, :], pt[:])
                        ps = psp3.tile([128, H2], f32, tag="ps")
                        for kt in range(k2):
                            nc.tensor.matmul(
                                ps[:],
                                lhsT=h2T[:, kt, :],
                                rhs=wg2_s[:, kt * H2 : (kt + 1) * H2],
                                start=(kt == 0),
                                stop=(kt == k2 - 1),
                            )
                        g2tile = tp3.tile([128, H2], bf, tag="g2")
                        nc.vector.tensor_copy(g2tile[:], ps[:])
                        nc.sync.dma_start(
                            g2_local[b * 128 : (b + 1) * 128, :], g2tile[:]
                        )

            nc.gpsimd.collective_compute(
                "AllGather",
                mybir.AluOpType.bypass,
                replica_groups=rg,
                ins=[g2_local[:, :]],
                outs=[g2_table[:, :]],
            )

            # ---------------- spmm2 + L4 fused per block
            with ExitStack() as ps2x:
                tps4 = ps2x.enter_context(
                    tc.tile_pool(name="tps4", bufs=2, space="PSUM")
                )
                psp4 = ps2x.enter_context(
                    tc.tile_pool(name="ps4", bufs=2, space="PSUM")
                )
                tp4 = ps2x.enter_context(tc.tile_pool(name="l4t", bufs=3))

                def cb2(b, psum):
                    h3t = tp4.tile([128, H2], bf, tag="h3")
                    nc.scalar.activation(h3t[:], psum[:], AF.Relu)
                    h3T = tp4.tile([128, k2, 128], bf, tag="h3T")
                    for kt in range(k2):
                        pt = tps4.tile([128, 128], bf, tag="pt")
                        nc.tensor.transpose(
                            pt[:], h3t[:, kt * 128 : (kt + 1) * 128], ident[:]
                        )
                        nc.vector.tensor_copy(h3T[:, kt, :], pt[:])
                    ps4 = psp4.tile([128, OUT], f32, tag="ps")
                    for kt in range(k2):
                        nc.tensor.matmul(
                            ps4[:],
                            lhsT=h3T[:, kt, :],
                            rhs=wl2_s[:, kt * OUT : (kt + 1) * OUT],
                            start=(kt == 0),
                            stop=False,
                        )
                    nc.tensor.matmul(
                        ps4[:],
                        lhsT=ones_t[:1, :],
                        rhs=bl2_s[:1, :],
                        start=False,
                        stop=True,
                    )
                    yt = tp4.tile([128, OUT], f32, tag="y")
                    nc.vector.tensor_copy(yt[:], ps4[:])
                    nc.sync.dma_start(y_d[b * 128 : (b + 1) * 128, :], yt[:])

                spmm(tc, ps2x, nc, g2_table, idx_s, ones_t, bg2_s, "b", cb2)

    nc.compile()
    return nc


# ---------------------------------------------------------------- driver

_CACHE = {}


def run(inputs, cfg: Cfg = FULL, trace=False, tmpdir=None):
    meta, in_maps = prep_inputs(cfg, inputs)
    key = (cfg, meta["totch"], meta["idxcols"])
    if key not in _CACHE:
        _CACHE[key] = build(cfg, meta)
    nc = _CACHE[key]
    res = run_bass_kernel_spmd(
        nc,
        in_maps,
        core_ids=list(range(cfg.n_cores)),
        trace=trace,
        tmpdir=tmpdir,
    )
    npc = cfg.nodes_per_core
    out = np.empty((cfg.n_nodes, cfg.out_dim), np.float32)
    for c in range(cfg.n_cores):
        lo = c * npc
        hi = min((c + 1) * npc, cfg.n_nodes)
        out[lo:hi] = res.results[c]["y"][: hi - lo]
    return out, res


def kernel(**inputs) -> np.ndarray:
    out, _ = run(inputs, FULL, trace=False)
    return out



# revision 6
# speedup vs baseline: 1.3771x; 1.3771x over previous
"""GNN message-passing kernel for 8 Trainium2 NeuronCores (Bass/Tile).

v2: gathers rotate across the 4 SWDGE queues (parallel Q7 desc-gen),
P-matrices generated on-chip (DVE iota-compare), node table split into
A/B halves with pipelined AllGathers, and phased (A-then-B) spmm
processing with bf16 partial flush so gather DMA prefetches deeply.
"""

import math
from contextlib import ExitStack
from dataclasses import dataclass

import ml_dtypes
import numpy as np

import concourse.bass as bass
import concourse.mybir as mybir
import concourse.tile as tile
from concourse import bacc
from concourse.bass_utils import run_bass_kernel_spmd
from concourse.masks import make_identity

BF16 = ml_dtypes.bfloat16
AF = mybir.ActivationFunctionType
ALU = mybir.AluOpType


@dataclass(frozen=True)
class Cfg:
    n_nodes: int = 50000
    n_edges: int = 800000
    in_dim: int = 512
    h1: int = 512
    h2: int = 256
    out_dim: int = 128
    n_cores: int = 8
    group_blocks: int = 3  # row-blocks per gather group

    @property
    def nodes_per_core(self):
        return math.ceil(self.n_nodes / self.n_cores)

    @property
    def npad(self):  # per-core padded nodes
        return math.ceil(self.nodes_per_core / 128) * 128

    @property
    def nblocks(self):
        return self.npad // 128

    @property
    def blocksA(self):  # blocks in table A (per-rank row half)
        return (self.nblocks + 1) // 2

    @property
    def rowsA(self):
        return self.blocksA * 128

    @property
    def rowsB(self):
        return self.npad - self.rowsA

    @property
    def ngroups(self):
        return math.ceil(self.nblocks / self.group_blocks)


FULL = Cfg()


# ---------------------------------------------------------------- host prep


def edge_structure(cfg: Cfg, edge_row, edge_col, edge_weight):
    """Per-core edge streams with SPMD-uniform chunk counts.

    Chunk order (global): table t (A=0,B=1) -> group g -> block b -> chunk.
    meta:
      nch[t][g]        chunks in gather call (table t, group g)
      chunk_blk[t][g]  block id per chunk (call order)
      off16[t][g]      idx-tile column offset (int16 cols)
      offch[t][g]      global chunk index offset
      totch, idxcols
    per_core[c]: idx [128, idxcols] i16; lr/w [128, totch] bf16
    """
    nc_, npad, nb, G = cfg.n_cores, cfg.npad, cfg.nblocks, cfg.group_blocks
    npc = cfg.nodes_per_core
    bA, rA, rB = cfg.blocksA, cfg.rowsA, cfg.rowsB
    assert max(rA, rB) * nc_ <= 32767, "tables must fit int16 indices"

    core_of = edge_row // npc
    lr_all = edge_row - core_of * npc
    cc = edge_col // npc
    cl = edge_col - cc * npc
    tb_all = (cl >= rA).astype(np.int64)
    tr_all = np.where(tb_all == 1, cc * rB + (cl - rA), cc * rA + cl)
    blk_all = lr_all // 128

    counts = np.zeros((nc_, nb, 2), np.int64)
    np.add.at(counts, (core_of, blk_all, tb_all), 1)
    chunks_bt = np.ceil(counts / 128.0).astype(np.int64).max(axis=0)  # [nb,2]

    ngroups = cfg.ngroups
    nch = [[0] * ngroups, [0] * ngroups]
    chunk_blk = [[[] for _ in range(ngroups)] for _ in range(2)]
    off16 = [[0] * ngroups, [0] * ngroups]
    offch = [[0] * ngroups, [0] * ngroups]
    tot16 = 0
    totch = 0
    for t in (0, 1):
        for g in range(ngroups):
            blocks = range(g * G, min((g + 1) * G, nb))
            off16[t][g] = tot16
            offch[t][g] = totch
            n = 0
            for b in blocks:
                cb_ = int(chunks_bt[b, t])
                chunk_blk[t][g].extend([b] * cb_)
                n += cb_
            nch[t][g] = n
            tot16 += n * 8
            totch += n

    meta = dict(
        nch=nch,
        chunk_blk=chunk_blk,
        off16=off16,
        offch=offch,
        totch=totch,
        idxcols=max(tot16, 8),
        chunks_bt=chunks_bt,
    )

    per_core = []
    for c in range(nc_):
        m = core_of == c
        lrc, trc, tbc, wc, blkc = (
            lr_all[m],
            tr_all[m],
            tb_all[m],
            edge_weight[m],
            blk_all[m],
        )
        idx_flat = np.zeros(meta["idxcols"] * 16, np.int16)
        lr_arr = np.full((128, totch), 255.0, np.float32)  # sentinel: no match
        w_arr = np.zeros((128, totch), np.float32)
        for t in (0, 1):
            for g in range(ngroups):
                blocks = range(g * G, min((g + 1) * G, nb))
                jc = offch[t][g]
                base16 = off16[t][g]
                i_call = 0
                for b in blocks:
                    cb_ = int(chunks_bt[b, t])
                    if cb_ == 0:
                        continue
                    sel = (blkc == b) & (tbc == t)
                    e_tr = trc[sel]
                    e_lr = lrc[sel]
                    e_w = wc[sel]
                    ne = len(e_tr)
                    vals = np.zeros(cb_ * 128, np.int64)
                    vals[:ne] = e_tr
                    ii = np.arange(cb_ * 128) + i_call
                    idx_flat[(base16 + ii // 16) * 16 + (ii % 16)] = vals.astype(
                        np.int16
                    )
                    i_call += cb_ * 128
                    if ne:
                        i_in = np.arange(ne)
                        lr_arr[i_in % 128, jc + i_in // 128] = (
                            e_lr - b * 128
                        ).astype(np.float32)
                        w_arr[i_in % 128, jc + i_in // 128] = e_w.astype(
                            np.float32
                        )
                    jc += cb_
        idx_mat = np.tile(idx_flat.reshape(meta["idxcols"], 16).T, (8, 1))
        per_core.append(
            dict(
                idx=np.ascontiguousarray(idx_mat),
                lr=np.ascontiguousarray(lr_arr),
                w=np.ascontiguousarray(w_arr),
            )
        )
    return meta, per_core


def prep_inputs(cfg: Cfg, inputs):
    """Shard + lay out all per-core input tensors."""
    f = inputs["features"].astype(np.float32)
    meta, per_edge = edge_structure(
        cfg,
        inputs["edge_row"].astype(np.int64),
        inputs["edge_col"].astype(np.int64),
        inputs["edge_weight"].astype(np.float32),
    )
    kin = cfg.in_dim // 128
    k1 = cfg.h1 // 128
    k2 = cfg.h2 // 128

    def wlayout(w, kt):
        K, M = w.shape
        return (
            w.reshape(kt, 128, M).transpose(1, 0, 2).reshape(128, kt * M)
        ).astype(BF16)

    w1 = wlayout(inputs["W_lin1"].astype(np.float32), kin)
    wg1 = wlayout(inputs["W_g1"].astype(np.float32), k1)
    wg2 = wlayout(inputs["W_g2"].astype(np.float32), k2)
    wl2 = wlayout(inputs["W_lin2"].astype(np.float32), k2)
    b1 = inputs["b_lin1"].astype(np.float32).reshape(kin, 128).T.copy()
    bg1 = inputs["b_g1"].astype(BF16).reshape(1, cfg.h2)
    bg2 = inputs["b_g2"].astype(BF16).reshape(1, cfg.h2)
    bl2 = inputs["b_lin2"].astype(BF16).reshape(1, cfg.out_dim)
    iota = np.tile(np.arange(128, dtype=np.float32), (128, 1)).astype(BF16)

    npc, npad = cfg.nodes_per_core, cfg.npad
    in_maps = []
    for c in range(cfg.n_cores):
        lo = c * npc
        hi = min((c + 1) * npc, cfg.n_nodes)
        xc = np.zeros((npad, cfg.in_dim), np.float32)
        xc[: hi - lo] = f[lo:hi]
        xt = (
            xc.T.reshape(kin, 128, npad)
            .transpose(1, 0, 2)
            .reshape(128, kin * npad)
        ).astype(BF16)
        in_maps.append(
            {
                "xt": np.ascontiguousarray(xt),
                "w1": w1,
                "wg1": wg1,
                "wg2": wg2,
                "wl2": wl2,
                "b1": b1,
                "bg1": bg1,
                "bg2": bg2,
                "bl2": bl2,
                "iota": iota,
                "idx": per_edge[c]["idx"],
                "lrv": per_edge[c]["lr"],
                "wv": per_edge[c]["w"],
            }
        )
    return meta, in_maps


# ---------------------------------------------------------------- kernel IR


def build(cfg: Cfg, meta):
    nc = bacc.Bacc(
        "TRN2",
        target_bir_lowering=False,
        debug=False,
        num_devices=cfg.n_cores,
        num_swdge_queues=4,
    )
    bf = mybir.dt.bfloat16
    f32 = mybir.dt.float32
    i16 = mybir.dt.int16
    kin = cfg.in_dim // 128
    k1 = cfg.h1 // 128
    k2 = cfg.h2 // 128
    npad, nb, G, H2, OUT = (
        cfg.npad,
        cfg.nblocks,
        cfg.group_blocks,
        cfg.h2,
        cfg.out_dim,
    )
    bA, rA, rB = cfg.blocksA, cfg.rowsA, cfg.rowsB
    totch = meta["totch"]
    ngroups = cfg.ngroups

    xt_d = nc.dram_tensor("xt", [128, kin * npad], bf, kind="ExternalInput").ap()
    w1_d = nc.dram_tensor("w1", [128, kin * cfg.h1], bf, kind="ExternalInput").ap()
    wg1_d = nc.dram_tensor("wg1", [128, k1 * H2], bf, kind="ExternalInput").ap()
    wg2_d = nc.dram_tensor("wg2", [128, k2 * H2], bf, kind="ExternalInput").ap()
    wl2_d = nc.dram_tensor("wl2", [128, k2 * OUT], bf, kind="ExternalInput").ap()
    b1_d = nc.dram_tensor("b1", [128, kin], f32, kind="ExternalInput").ap()
    bg1_d = nc.dram_tensor("bg1", [1, H2], bf, kind="ExternalInput").ap()
    bg2_d = nc.dram_tensor("bg2", [1, H2], bf, kind="ExternalInput").ap()
    bl2_d = nc.dram_tensor("bl2", [1, OUT], bf, kind="ExternalInput").ap()
    iota_d = nc.dram_tensor("iota", [128, 128], bf, kind="ExternalInput").ap()
    idx_d = nc.dram_tensor(
        "idx", [128, meta["idxcols"]], i16, kind="ExternalInput"
    ).ap()
    lr_d = nc.dram_tensor("lrv", [128, totch], f32, kind="ExternalInput").ap()
    wv_d = nc.dram_tensor("wv", [128, totch], f32, kind="ExternalInput").ap()
    y_d = nc.dram_tensor("y", [npad, OUT], f32, kind="ExternalOutput").ap()

    g1_localA = nc.dram_tensor("g1_localA", [rA, H2], bf).ap()
    g1_localB = nc.dram_tensor("g1_localB", [rB, H2], bf).ap()
    g2_localA = nc.dram_tensor("g2_localA", [rA, H2], bf).ap()
    g2_localB = nc.dram_tensor("g2_localB", [rB, H2], bf).ap()
    t1A = nc.dram_tensor("t1A", [rA * cfg.n_cores, H2], bf, addr_space="Shared").ap()
    t1B = nc.dram_tensor("t1B", [rB * cfg.n_cores, H2], bf, addr_space="Shared").ap()
    t2A = nc.dram_tensor("t2A", [rA * cfg.n_cores, H2], bf, addr_space="Shared").ap()
    t2B = nc.dram_tensor("t2B", [rB * cfg.n_cores, H2], bf, addr_space="Shared").ap()

    rg = [list(range(cfg.n_cores))]
    qstate = [0]

    def next_q():
        q = qstate[0]
        qstate[0] = (q + 1) % 4
        return q

    # last-chunk flag per (t, block): global chunk index of the final chunk
    last_chunk = {}
    for t in (0, 1):
        for g in range(ngroups):
            for j, b in enumerate(meta["chunk_blk"][t][g]):
                last_chunk[(t, b)] = meta["offch"][t][g] + j

    def spmm(tc, ctx, nc, tabs, idx_s, lr_s, wv_s, iota_s, ident, ones_t,
             brow, tag, out_cb, post_group=None):
        """Phased weighted segment-sum: all A-table work, then all B."""
        gp = ctx.enter_context(tc.tile_pool(name=f"g{tag}", bufs=4))
        pp = ctx.enter_context(tc.tile_pool(name=f"pm{tag}", bufs=4))
        sp = ctx.enter_context(
            tc.tile_pool(name=f"ps{tag}", bufs=G + 1, space="PSUM")
        )
        pap = ctx.enter_context(tc.tile_pool(name=f"pa{tag}", bufs=1))
        partial = pap.tile([128, nb * H2], bf)

        for t in (0, 1):
            for g in range(ngroups):
                blocks = list(range(g * G, min((g + 1) * G, nb)))
                n = meta["nch"][t][g]
                gt = None
                pt_ = None
                if n:
                    gt = gp.tile([128, n, H2], bf, tag="g")
                    for lo in range(0, n, 15):
                        ns = min(15, n - lo)
                        o16 = meta["off16"][t][g] + lo * 8
                        nc.gpsimd.dma_gather(
                            out_ap=gt[:, lo : lo + ns, :],
                            in_ap=tabs[t][:, :],
                            idxs_ap=idx_s[:, o16 : o16 + ns * 8],
                            num_idxs=ns * 128,
                            num_idxs_reg=ns * 128,
                            elem_size=H2,
                            single_packet=False,
                            queue_num=next_q(),
                        )
                    pt_ = pp.tile([128, n * 128], bf, tag="p")
                    for j in range(n):
                        jc = meta["offch"][t][g] + j
                        nc.vector.tensor_scalar(
                            pt_[:, j * 128 : (j + 1) * 128],
                            iota_s[:, :],
                            lr_s[:, jc : jc + 1],
                            wv_s[:, jc : jc + 1],
                            ALU.is_equal,
                            ALU.mult,
                        )
                psums = {
                    b: sp.tile([128, H2], f32, tag="ps", name=f"ps{tag}_{b % (G+1)}")
                    for b in blocks
                }
                for b in blocks:
                    empty = (t, b) not in last_chunk
                    if t == 0:
                        nc.tensor.matmul(
                            psums[b][:],
                            lhsT=ones_t[:1, :],
                            rhs=brow[:1, :],
                            start=True,
                            stop=empty,
                        )
                    else:
                        nc.tensor.matmul(
                            psums[b][:],
                            lhsT=ident[:, :],
                            rhs=partial[:, b * H2 : (b + 1) * H2],
                            start=True,
                            stop=empty,
                        )
                for j, b in enumerate(meta["chunk_blk"][t][g]):
                    jc = meta["offch"][t][g] + j
                    nc.tensor.matmul(
                        psums[b][:],
                        lhsT=pt_[:, j * 128 : (j + 1) * 128],
                        rhs=gt[:, j, :],
                        start=False,
                        stop=(jc == last_chunk[(t, b)]),
                    )
                for b in blocks:
                    if t == 0:
                        nc.vector.tensor_copy(
                            partial[:, b * H2 : (b + 1) * H2], psums[b][:]
                        )
                    else:
                        out_cb(b, psums[b])
                if post_group is not None:
                    post_group(t, g)

    with tile.TileContext(nc) as tc:
        with ExitStack() as top:
            const = top.enter_context(tc.tile_pool(name="const", bufs=1))
            w1_s = const.tile([128, kin * cfg.h1], bf)
            nc.sync.dma_start(w1_s[:], w1_d[:, :])
            wg1_s = const.tile([128, k1 * H2], bf)
            nc.sync.dma_start(wg1_s[:], wg1_d[:, :])
            wg2_s = const.tile([128, k2 * H2], bf)
            nc.sync.dma_start(wg2_s[:], wg2_d[:, :])
            wl2_s = const.tile([128, k2 * OUT], bf)
            nc.sync.dma_start(wl2_s[:], wl2_d[:, :])
            b1_s = const.tile([128, kin], f32)
            nc.sync.dma_start(b1_s[:], b1_d[:, :])
            bg1_s = const.tile([1, H2], bf)
            nc.sync.dma_start(bg1_s[:], bg1_d[:, :])
            bg2_s = const.tile([1, H2], bf)
            nc.sync.dma_start(bg2_s[:], bg2_d[:, :])
            bl2_s = const.tile([1, OUT], bf)
            nc.sync.dma_start(bl2_s[:], bl2_d[:, :])
            iota_s = const.tile([128, 128], bf)
            nc.sync.dma_start(iota_s[:], iota_d[:, :])
            idx_s = const.tile([128, meta["idxcols"]], i16)
            nc.sync.dma_start(idx_s[:], idx_d[:, :])
            lr_s = const.tile([128, totch], f32)
            nc.sync.dma_start(lr_s[:], lr_d[:, :])
            wv_s = const.tile([128, totch], f32)
            nc.sync.dma_start(wv_s[:], wv_d[:, :])
            ident = const.tile([128, 128], bf)
            make_identity(nc, ident[:])
            ones_t = const.tile([1, 128], bf)
            nc.gpsimd.memset(ones_t[:], 1.0)

            # ---------------- L1 + L2a, A half then B half, AG after each
            with ExitStack() as s1:
                h1p = s1.enter_context(tc.tile_pool(name="h1t", bufs=1))
                h1t = h1p.tile([128, k1 * npad], bf)
                xp = s1.enter_context(tc.tile_pool(name="xt", bufs=1))
                xt_s = xp.tile([128, kin * npad], bf)
                psp = s1.enter_context(
                    tc.tile_pool(name="ps1", bufs=4, space="PSUM")
                )
                psp2 = s1.enter_context(
                    tc.tile_pool(name="ps2", bufs=4, space="PSUM")
                )
                tp2 = s1.enter_context(tc.tile_pool(name="g1t", bufs=3))

                halves = [
                    (0, rA, range(0, bA), g1_localA, t1A),
                    (rA, npad, range(bA, nb), g1_localB, t1B),
                ]
                for lo_n, hi_n, blocks, gl, tab in halves:
                    for kt in range(kin):
                        nc.sync.dma_start(
                            xt_s[:, kt * npad + lo_n : kt * npad + hi_n],
                            xt_d[:, kt * npad + lo_n : kt * npad + hi_n],
                        )
                    nsl = [
                        (a, min(a + 512, hi_n))
                        for a in range(lo_n, hi_n, 512)
                    ]
                    for f1t in range(k1):
                        for a, b_ in nsl:
                            nw = b_ - a
                            ps = psp.tile([128, 512], f32, tag="ps")
                            for kt in range(kin):
                                nc.tensor.matmul(
                                    ps[:, :nw],
                                    lhsT=w1_s[
                                        :,
                                        kt * cfg.h1
                                        + f1t * 128 : kt * cfg.h1
                                        + f1t * 128
                                        + 128,
                                    ],
                                    rhs=xt_s[:, kt * npad + a : kt * npad + b_],
                                    start=(kt == 0),
                                    stop=(kt == kin - 1),
                                )
                            nc.scalar.activation(
                                h1t[:, f1t * npad + a : f1t * npad + b_],
                                ps[:, :nw],
                                AF.Sigmoid,
                                bias=b1_s[:, f1t : f1t + 1],
                            )
                    for b in blocks:
                        ps = psp2.tile([128, H2], f32, tag="ps")
                        for kt in range(k1):
                            nc.tensor.matmul(
                                ps[:],
                                lhsT=h1t[
                                    :,
                                    kt * npad + b * 128 : kt * npad + b * 128 + 128,
                                ],
                                rhs=wg1_s[:, kt * H2 : (kt + 1) * H2],
                                start=(kt == 0),
                                stop=(kt == k1 - 1),
                            )
                        g1tile = tp2.tile([128, H2], bf, tag="g1")
                        nc.vector.tensor_copy(g1tile[:], ps[:])
                        roff = b * 128 - lo_n
                        nc.sync.dma_start(
                            gl[roff : roff + 128, :], g1tile[:]
                        )
                    nc.gpsimd.collective_compute(
                        "AllGather",
                        ALU.bypass,
                        replica_groups=rg,
                        ins=[gl[:, :]],
                        outs=[tab[:, :]],
                    )

            # ---------------- spmm1 fused with L3a; AG2 halves pipelined
            agdone = [False, False]

            with ExitStack() as s2:
                tps = s2.enter_context(
                    tc.tile_pool(name="tps", bufs=2, space="PSUM")
                )
                psp3 = s2.enter_context(
                    tc.tile_pool(name="ps3", bufs=2, space="PSUM")
                )
                tp3 = s2.enter_context(tc.tile_pool(name="l3t", bufs=3))

                def cb1(b, psum):
                    h3t = tp3.tile([128, H2], bf, tag="h3")
                    nc.scalar.activation(h3t[:], psum[:], AF.Relu)
                    h2T = tp3.tile([128, k2, 128], bf, tag="h2T")
                    for kt in range(k2):
                        ptt = tps.tile([128, 128], bf, tag="pt")
                        nc.tensor.transpose(
                            ptt[:],
                            h3t[:, kt * 128 : (kt + 1) * 128],
                            ident[:],
                        )
                        nc.vector.tensor_copy(h2T[:, kt, :], ptt[:])
                    ps3 = psp3.tile([128, H2], f32, tag="ps3")
                    for kt in range(k2):
                        nc.tensor.matmul(
                            ps3[:],
                            lhsT=h2T[:, kt, :],
                            rhs=wg2_s[:, kt * H2 : (kt + 1) * H2],
                            start=(kt == 0),
                            stop=(kt == k2 - 1),
                        )
                    g2t = tp3.tile([128, H2], bf, tag="g2")
                    nc.vector.tensor_copy(g2t[:], ps3[:])
                    if b < bA:
                        nc.sync.dma_start(
                            g2_localA[b * 128 : b * 128 + 128, :], g2t[:]
                        )
                    else:
                        roff = b * 128 - rA
                        nc.sync.dma_start(
                            g2_localB[roff : roff + 128, :], g2t[:]
                        )

                def post1(t, g):
                    # AG2-A once blocks 0..bA-1 are produced (phase B)
                    hi_blk = min((g + 1) * G, nb) - 1
                    if t == 1 and not agdone[0] and hi_blk >= bA - 1:
                        nc.gpsimd.collective_compute(
                            "AllGather",
                            ALU.bypass,
                            replica_groups=rg,
                            ins=[g2_localA[:, :]],
                            outs=[t2A[:, :]],
                        )
                        agdone[0] = True

                spmm(
                    tc, s2, nc, (t1A, t1B), idx_s, lr_s, wv_s, iota_s,
                    ident, ones_t, bg1_s, "a", cb1, post1,
                )

            nc.gpsimd.collective_compute(
                "AllGather",
                ALU.bypass,
                replica_groups=rg,
                ins=[g2_localB[:, :]],
                outs=[t2B[:, :]],
            )

            # ---------------- spmm2 fused with L4
            with ExitStack() as s3:
                tps4 = s3.enter_context(
                    tc.tile_pool(name="tps4", bufs=2, space="PSUM")
                )
                psp4 = s3.enter_context(
                    tc.tile_pool(name="ps4", bufs=2, space="PSUM")
                )
                tp4 = s3.enter_context(tc.tile_pool(name="l4t", bufs=3))

                def cb2(b, psum):
                    h3t = tp4.tile([128, H2], bf, tag="h3")
                    nc.scalar.activation(h3t[:], psum[:], AF.Relu)
                    h3T = tp4.tile([128, k2, 128], bf, tag="h3T")
                    for kt in range(k2):
                        ptt = tps4.tile([128, 128], bf, tag="pt")
                        nc.tensor.transpose(
                            ptt[:], h3t[:, kt * 128 : (kt + 1) * 128], ident[:]
                        )
                        nc.vector.tensor_copy(h3T[:, kt, :], ptt[:])
                    ps4 = psp4.tile([128, OUT], f32, tag="ps")
                    for kt in range(k2):
                        nc.tensor.matmul(
                            ps4[:],
                            lhsT=h3T[:, kt, :],
                            rhs=wl2_s[:, kt * OUT : (kt + 1) * OUT],
                            start=(kt == 0),
                            stop=False,
                        )
                    nc.tensor.matmul(
                        ps4[:],
                        lhsT=ones_t[:1, :],
                        rhs=bl2_s[:1, :],
                        start=False,
                        stop=True,
                    )
                    yt = tp4.tile([128, OUT], f32, tag="y")
                    nc.vector.tensor_copy(yt[:], ps4[:])
                    nc.sync.dma_start(y_d[b * 128 : (b + 1) * 128, :], yt[:])

                spmm(
                    tc, s3, nc, (t2A, t2B), idx_s, lr_s, wv_s, iota_s,
                    ident, ones_t, bg2_s, "b", cb2,
                )

    nc.compile()
    return nc


# ---------------------------------------------------------------- driver

_CACHE = {}


def run(inputs, cfg: Cfg = FULL, trace=False, tmpdir=None):
    meta, in_maps = prep_inputs(cfg, inputs)
    key = (cfg, meta["totch"], meta["idxcols"])
    if key not in _CACHE:
        _CACHE[key] = build(cfg, meta)
    nc = _CACHE[key]
    res = run_bass_kernel_spmd(
        nc,
        in_maps,
        core_ids=list(range(cfg.n_cores)),
        trace=trace,
        tmpdir=tmpdir,
    )
    npc = cfg.nodes_per_core
    out = np.empty((cfg.n_nodes, cfg.out_dim), np.float32)
    for c in range(cfg.n_cores):
        lo = c * npc
        hi = min((c + 1) * npc, cfg.n_nodes)
        out[lo:hi] = res.results[c]["y"][: hi - lo]
    return out, res


def kernel(**inputs) -> np.ndarray:
    out, _ = run(inputs, FULL, trace=False)
    return out


# revision 12
# speedup vs baseline: 1.4215x; 1.0323x over previous
"""GNN message-passing kernel for 8 Trainium2 NeuronCores (Bass/Tile).

v3: host-built one-hot P matrices streamed from HBM, flipped spmm
matmuls (gathered rows stationary, P streamed) producing transposed
psums so relu+bias run natively on the scalar engine and L3a/L4 need
no transposes; node table split A/B with pipelined AllGathers; phased
(A-then-B) spmm with bf16 partial flush; gathers rotate across 4 SWDGE
queues with an enlarged descriptor-ring carveout.
"""

import math
from contextlib import ExitStack
from dataclasses import dataclass

import ml_dtypes
import numpy as np

import concourse.bass as bass
import concourse.mybir as mybir
import concourse.tile as tile
from concourse import bacc
from concourse.bass_utils import run_bass_kernel_spmd
from concourse.masks import make_identity

BF16 = ml_dtypes.bfloat16
AF = mybir.ActivationFunctionType
ALU = mybir.AluOpType


@dataclass(frozen=True)
class Cfg:
    n_nodes: int = 50000
    n_edges: int = 800000
    in_dim: int = 512
    h1: int = 512
    h2: int = 256
    out_dim: int = 128
    n_cores: int = 8
    group_blocks: int = 3  # row-blocks per gather group

    @property
    def nodes_per_core(self):
        return math.ceil(self.n_nodes / self.n_cores)

    @property
    def npad(self):  # per-core padded nodes
        return math.ceil(self.nodes_per_core / 128) * 128

    @property
    def nblocks(self):
        return self.npad // 128

    @property
    def blocksA(self):  # blocks in table A (per-rank row half)
        return (self.nblocks + 1) // 2

    @property
    def rowsA(self):
        return self.blocksA * 128

    @property
    def rowsB(self):
        return self.npad - self.rowsA

    @property
    def ngroups(self):
        return math.ceil(self.nblocks / self.group_blocks)


FULL = Cfg()


# ---------------------------------------------------------------- host prep


def edge_structure(cfg: Cfg, edge_row, edge_col, edge_weight):
    """Per-core edge streams with SPMD-uniform chunk counts.

    Chunk order (global): table t (A=0,B=1) -> group g -> block b -> chunk.
    per_core[c]: idx [128, idxcols] i16; pmat [128, totch*128] bf16 with
    pmat[slot, jc*128 + r] = w for the edge at (chunk jc, slot) with local
    destination row r.
    """
    nc_, npad, nb, G = cfg.n_cores, cfg.npad, cfg.nblocks, cfg.group_blocks
    npc = cfg.nodes_per_core
    bA, rA, rB = cfg.blocksA, cfg.rowsA, cfg.rowsB
    assert max(rA, rB) * nc_ <= 32767, "tables must fit int16 indices"

    core_of = edge_row // npc
    lr_all = edge_row - core_of * npc
    cc = edge_col // npc
    cl = edge_col - cc * npc
    tb_all = (cl >= rA).astype(np.int64)
    tr_all = np.where(tb_all == 1, cc * rB + (cl - rA), cc * rA + cl)
    blk_all = lr_all // 128

    counts = np.zeros((nc_, nb, 2), np.int64)
    np.add.at(counts, (core_of, blk_all, tb_all), 1)
    chunks_bt = np.ceil(counts / 128.0).astype(np.int64).max(axis=0)  # [nb,2]
    chunks_bt = np.maximum(chunks_bt, 1)  # every (block, table) has >=1 chunk

    ngroups = cfg.ngroups
    nch = [[0] * ngroups, [0] * ngroups]
    chunk_blk = [[[] for _ in range(ngroups)] for _ in range(2)]
    off16 = [[0] * ngroups, [0] * ngroups]
    offch = [[0] * ngroups, [0] * ngroups]
    tot16 = 0
    totch = 0
    for t in (0, 1):
        for g in range(ngroups):
            blocks = range(g * G, min((g + 1) * G, nb))
            off16[t][g] = tot16
            offch[t][g] = totch
            n = 0
            for b in blocks:
                cb_ = int(chunks_bt[b, t])
                chunk_blk[t][g].extend([b] * cb_)
                n += cb_
            nch[t][g] = n
            tot16 += n * 8
            totch += n

    meta = dict(
        nch=nch,
        chunk_blk=chunk_blk,
        off16=off16,
        offch=offch,
        totch=totch,
        idxcols=max(tot16, 8),
        chunks_bt=chunks_bt,
    )

    per_core = []
    for c in range(nc_):
        m = core_of == c
        lrc, trc, tbc, wc, blkc = (
            lr_all[m],
            tr_all[m],
            tb_all[m],
            edge_weight[m],
            blk_all[m],
        )
        idx_flat = np.zeros(meta["idxcols"] * 16, np.int16)
        pmat = np.zeros((128, totch * 128), BF16)
        for t in (0, 1):
            for g in range(ngroups):
                blocks = range(g * G, min((g + 1) * G, nb))
                jc = offch[t][g]
                base16 = off16[t][g]
                i_call = 0
                for b in blocks:
                    cb_ = int(chunks_bt[b, t])
                    sel = (blkc == b) & (tbc == t)
                    e_tr = trc[sel]
                    e_lr = lrc[sel]
                    e_w = wc[sel]
                    ne = len(e_tr)
                    vals = np.zeros(cb_ * 128, np.int64)
                    vals[:ne] = e_tr
                    ii = np.arange(cb_ * 128) + i_call
                    idx_flat[(base16 + ii // 16) * 16 + (ii % 16)] = vals.astype(
                        np.int16
                    )
                    i_call += cb_ * 128
                    if ne:
                        i_in = np.arange(ne)
                        jj = jc + i_in // 128
                        slot = i_in % 128
                        r = e_lr - b * 128
                        pmat[slot, jj * 128 + r] = e_w.astype(BF16)
                    jc += cb_
        idx_mat = np.tile(idx_flat.reshape(meta["idxcols"], 16).T, (8, 1))
        per_core.append(
            dict(
                idx=np.ascontiguousarray(idx_mat),
                pmat=pmat,
            )
        )
    return meta, per_core


def prep_inputs(cfg: Cfg, inputs):
    """Shard + lay out all per-core input tensors."""
    f = inputs["features"].astype(np.float32)
    meta, per_edge = edge_structure(
        cfg,
        inputs["edge_row"].astype(np.int64),
        inputs["edge_col"].astype(np.int64),
        inputs["edge_weight"].astype(np.float32),
    )
    kin = cfg.in_dim // 128
    k1 = cfg.h1 // 128
    k2 = cfg.h2 // 128

    def wlayout(w, kt):
        K, M = w.shape
        return (
            w.reshape(kt, 128, M).transpose(1, 0, 2).reshape(128, kt * M)
        ).astype(BF16)

    w1 = wlayout(inputs["W_lin1"].astype(np.float32), kin)
    wg1 = wlayout(inputs["W_g1"].astype(np.float32), k1)
    wg2 = wlayout(inputs["W_g2"].astype(np.float32), k2)
    wl2 = wlayout(inputs["W_lin2"].astype(np.float32), k2)
    b1 = inputs["b_lin1"].astype(np.float32).reshape(kin, 128).T.copy()
    # per-partition bias columns for the transposed spmm outputs
    bg1 = inputs["b_g1"].astype(np.float32).reshape(k2, 128).T.copy()
    bg2 = inputs["b_g2"].astype(np.float32).reshape(k2, 128).T.copy()
    bl2 = inputs["b_lin2"].astype(BF16).reshape(1, cfg.out_dim)

    npc, npad = cfg.nodes_per_core, cfg.npad
    in_maps = []
    for c in range(cfg.n_cores):
        lo = c * npc
        hi = min((c + 1) * npc, cfg.n_nodes)
        xc = np.zeros((npad, cfg.in_dim), np.float32)
        xc[: hi - lo] = f[lo:hi]
        xt = (
            xc.T.reshape(kin, 128, npad)
            .transpose(1, 0, 2)
            .reshape(128, kin * npad)
        ).astype(BF16)
        in_maps.append(
            {
                "xt": np.ascontiguousarray(xt),
                "w1": w1,
                "wg1": wg1,
                "wg2": wg2,
                "wl2": wl2,
                "b1": b1,
                "bg1": bg1,
                "bg2": bg2,
                "bl2": bl2,
                "idx": per_edge[c]["idx"],
                "pmat": per_edge[c]["pmat"],
            }
        )
    return meta, in_maps


# ---------------------------------------------------------------- kernel IR


def build(cfg: Cfg, meta):
    nc = bacc.Bacc(
        "TRN2",
        target_bir_lowering=False,
        debug=False,
        num_devices=cfg.n_cores,
        num_swdge_queues=4,
    )
    bf = mybir.dt.bfloat16
    f32 = mybir.dt.float32
    i16 = mybir.dt.int16
    kin = cfg.in_dim // 128
    k1 = cfg.h1 // 128
    k2 = cfg.h2 // 128
    npad, nb, G, H2, OUT = (
        cfg.npad,
        cfg.nblocks,
        cfg.group_blocks,
        cfg.h2,
        cfg.out_dim,
    )
    bA, rA, rB = cfg.blocksA, cfg.rowsA, cfg.rowsB
    totch = meta["totch"]
    ngroups = cfg.ngroups

    xt_d = nc.dram_tensor("xt", [128, kin * npad], bf, kind="ExternalInput").ap()
    w1_d = nc.dram_tensor("w1", [128, kin * cfg.h1], bf, kind="ExternalInput").ap()
    wg1_d = nc.dram_tensor("wg1", [128, k1 * H2], bf, kind="ExternalInput").ap()
    wg2_d = nc.dram_tensor("wg2", [128, k2 * H2], bf, kind="ExternalInput").ap()
    wl2_d = nc.dram_tensor("wl2", [128, k2 * OUT], bf, kind="ExternalInput").ap()
    b1_d = nc.dram_tensor("b1", [128, kin], f32, kind="ExternalInput").ap()
    bg1_d = nc.dram_tensor("bg1", [128, k2], f32, kind="ExternalInput").ap()
    bg2_d = nc.dram_tensor("bg2", [128, k2], f32, kind="ExternalInput").ap()
    bl2_d = nc.dram_tensor("bl2", [1, OUT], bf, kind="ExternalInput").ap()
    idx_d = nc.dram_tensor(
        "idx", [128, meta["idxcols"]], i16, kind="ExternalInput"
    ).ap()
    pmat_d = nc.dram_tensor(
        "pmat", [128, totch * 128], bf, kind="ExternalInput"
    ).ap()
    y_d = nc.dram_tensor("y", [npad, OUT], f32, kind="ExternalOutput").ap()

    g1_localA = nc.dram_tensor("g1_localA", [rA, H2], bf).ap()
    g1_localB = nc.dram_tensor("g1_localB", [rB, H2], bf).ap()
    g2_localA = nc.dram_tensor("g2_localA", [rA, H2], bf).ap()
    g2_localB = nc.dram_tensor("g2_localB", [rB, H2], bf).ap()
    t1A = nc.dram_tensor("t1A", [rA * cfg.n_cores, H2], bf, addr_space="Shared").ap()
    t1B = nc.dram_tensor("t1B", [rB * cfg.n_cores, H2], bf, addr_space="Shared").ap()
    t2A = nc.dram_tensor("t2A", [rA * cfg.n_cores, H2], bf, addr_space="Shared").ap()
    t2B = nc.dram_tensor("t2B", [rB * cfg.n_cores, H2], bf, addr_space="Shared").ap()

    rg = [list(range(cfg.n_cores))]
    qstate = [0]

    def next_q():
        q = qstate[0]
        qstate[0] = (q + 1) % 4
        return q

    # last global chunk index per (t, block)
    last_chunk = {}
    for t in (0, 1):
        for g in range(ngroups):
            for j, b in enumerate(meta["chunk_blk"][t][g]):
                last_chunk[(t, b)] = meta["offch"][t][g] + j
    def spmm(tc, ctx, nc, tabs, idx_s, ident, tag, out_cb, post_group=None):
        """Phased weighted segment-sum with transposed psums.

        psum half h of block b accumulates sum_e w_e * h[col_e, h*128:...]^T
        -> [feat 128, row 128]."""
        gp = ctx.enter_context(tc.tile_pool(name=f"g{tag}", bufs=4))
        pp = ctx.enter_context(tc.tile_pool(name=f"pm{tag}", bufs=3))
        sp = ctx.enter_context(
            tc.tile_pool(name=f"ps{tag}", bufs=G + 1, space="PSUM")
        )
        pap = ctx.enter_context(tc.tile_pool(name=f"pa{tag}", bufs=1))
        partial = pap.tile([128, nb * H2], bf)

        for t in (0, 1):
            for g in range(ngroups):
                blocks = list(range(g * G, min((g + 1) * G, nb)))
                n = meta["nch"][t][g]
                gt = gp.tile([128, n, H2], bf, tag="g")
                for lo in range(0, n, 15):
                    ns = min(15, n - lo)
                    o16 = meta["off16"][t][g] + lo * 8
                    nc.gpsimd.dma_gather(
                        out_ap=gt[:, lo : lo + ns, :],
                        in_ap=tabs[t][:, :],
                        idxs_ap=idx_s[:, o16 : o16 + ns * 8],
                        num_idxs=ns * 128,
                        num_idxs_reg=ns * 128,
                        elem_size=H2,
                        single_packet=False,
                        queue_num=next_q(),
                    )
                ptile = pp.tile([128, n * 128], bf, tag="p")
                nc.sync.dma_start(
                    ptile[:],
                    pmat_d[
                        :,
                        meta["offch"][t][g] * 128 : (meta["offch"][t][g] + n)
                        * 128,
                    ],
                )
                psums = {
                    b: sp.tile(
                        [128, H2], f32, tag="ps", name=f"ps{tag}_{b}"
                    )
                    for b in blocks
                }
                # single full-width start per block: start_tensor_calc
                # zeroes the whole 2KB psum bank, so per-half starts would
                # wipe each other
                for b in blocks:
                    if t == 0:
                        nc.tensor.matmul(
                            psums[b][:],
                            lhsT=ones_t[:1, :],
                            rhs=zrow[:1, :],
                            start=True,
                            stop=False,
                        )
                    else:
                        nc.tensor.matmul(
                            psums[b][:],
                            lhsT=ident[:, :],
                            rhs=partial[:, b * H2 : (b + 1) * H2],
                            start=True,
                            stop=False,
                        )
                for j, b in enumerate(meta["chunk_blk"][t][g]):
                    jc = meta["offch"][t][g] + j
                    for h in (0, 1):
                        nc.tensor.matmul(
                            psums[b][:, h * 128 : h * 128 + 128],
                            lhsT=gt[:, j, h * 128 : h * 128 + 128],
                            rhs=ptile[:, j * 128 : (j + 1) * 128],
                            start=False,
                            stop=(jc == last_chunk[(t, b)] and h == 1),
                        )
                for b in blocks:
                    if t == 0:
                        nc.vector.tensor_copy(
                            partial[:, b * H2 : (b + 1) * H2], psums[b][:]
                        )
                    else:
                        out_cb(b, psums[b])
                if post_group is not None:
                    post_group(t, g)

    with tile.TileContext(nc) as tc:
        with ExitStack() as top:
            const = top.enter_context(tc.tile_pool(name="const", bufs=1))
            w1_s = const.tile([128, kin * cfg.h1], bf)
            nc.sync.dma_start(w1_s[:], w1_d[:, :])
            wg1_s = const.tile([128, k1 * H2], bf)
            nc.sync.dma_start(wg1_s[:], wg1_d[:, :])
            wg2_s = const.tile([128, k2 * H2], bf)
            nc.sync.dma_start(wg2_s[:], wg2_d[:, :])
            wl2_s = const.tile([128, k2 * OUT], bf)
            nc.sync.dma_start(wl2_s[:], wl2_d[:, :])
            b1_s = const.tile([128, kin], f32)
            nc.sync.dma_start(b1_s[:], b1_d[:, :])
            bg1_s = const.tile([128, k2], f32)
            nc.sync.dma_start(bg1_s[:], bg1_d[:, :])
            bg2_s = const.tile([128, k2], f32)
            nc.sync.dma_start(bg2_s[:], bg2_d[:, :])
            bl2_s = const.tile([1, OUT], bf)
            nc.sync.dma_start(bl2_s[:], bl2_d[:, :])
            idx_s = const.tile([128, meta["idxcols"]], i16)
            nc.sync.dma_start(idx_s[:], idx_d[:, :])
            ident = const.tile([128, 128], bf)
            make_identity(nc, ident[:])
            ones_t = const.tile([1, 128], bf)
            nc.gpsimd.memset(ones_t[:], 1.0)
            zrow = const.tile([1, H2], bf)
            nc.gpsimd.memset(zrow[:], 0.0)

            # ---------------- L1 + L2a, A half then B half, AG after each
            with ExitStack() as s1:
                h1p = s1.enter_context(tc.tile_pool(name="h1t", bufs=1))
                h1t = h1p.tile([128, k1 * npad], bf)
                xp = s1.enter_context(tc.tile_pool(name="xt", bufs=1))
                xt_s = xp.tile([128, kin * npad], bf)
                psp = s1.enter_context(
                    tc.tile_pool(name="ps1", bufs=4, space="PSUM")
                )
                psp2 = s1.enter_context(
                    tc.tile_pool(name="ps2", bufs=4, space="PSUM")
                )
                tp2 = s1.enter_context(tc.tile_pool(name="g1t", bufs=3))

                halves = [
                    (0, rA, range(0, bA), g1_localA, t1A),
                    (rA, npad, range(bA, nb), g1_localB, t1B),
                ]
                for lo_n, hi_n, blocks, gl, tab in halves:
                    for kt in range(kin):
                        nc.sync.dma_start(
                            xt_s[:, kt * npad + lo_n : kt * npad + hi_n],
                            xt_d[:, kt * npad + lo_n : kt * npad + hi_n],
                        )
                    nsl = [
                        (a, min(a + 512, hi_n))
                        for a in range(lo_n, hi_n, 512)
                    ]
                    for f1t in range(k1):
                        for a, b_ in nsl:
                            nw = b_ - a
                            ps = psp.tile([128, 512], f32, tag="ps")
                            for kt in range(kin):
                                nc.tensor.matmul(
                                    ps[:, :nw],
                                    lhsT=w1_s[
                                        :,
                                        kt * cfg.h1
                                        + f1t * 128 : kt * cfg.h1
                                        + f1t * 128
                                        + 128,
                                    ],
                                    rhs=xt_s[:, kt * npad + a : kt * npad + b_],
                                    start=(kt == 0),
                                    stop=(kt == kin - 1),
                                )
                            nc.scalar.activation(
                                h1t[:, f1t * npad + a : f1t * npad + b_],
                                ps[:, :nw],
                                AF.Sigmoid,
                                bias=b1_s[:, f1t : f1t + 1],
                            )
                    for b in blocks:
                        ps = psp2.tile([128, H2], f32, tag="ps")
                        for kt in range(k1):
                            nc.tensor.matmul(
                                ps[:],
                                lhsT=h1t[
                                    :,
                                    kt * npad + b * 128 : kt * npad + b * 128 + 128,
                                ],
                                rhs=wg1_s[:, kt * H2 : (kt + 1) * H2],
                                start=(kt == 0),
                                stop=(kt == k1 - 1),
                            )
                        g1tile = tp2.tile([128, H2], bf, tag="g1")
                        nc.vector.tensor_copy(g1tile[:], ps[:])
                        roff = b * 128 - lo_n
                        nc.sync.dma_start(
                            gl[roff : roff + 128, :], g1tile[:]
                        )
                    nc.gpsimd.collective_compute(
                        "AllGather",
                        ALU.bypass,
                        replica_groups=rg,
                        ins=[gl[:, :]],
                        outs=[tab[:, :]],
                    )

            # ---------------- spmm1 fused with L3a; AG2 halves pipelined
            agdone = [False]

            with ExitStack() as s2:
                psp3 = s2.enter_context(
                    tc.tile_pool(name="ps3", bufs=2, space="PSUM")
                )
                tp3 = s2.enter_context(tc.tile_pool(name="l3t", bufs=3))

                def cb1(b, psT):
                    h2T = tp3.tile([128, k2, 128], bf, tag="h2T")
                    for kt in range(k2):
                        nc.scalar.activation(
                            h2T[:, kt, :],
                            psT[:, kt * 128 : kt * 128 + 128],
                            AF.Relu,
                            bias=bg1_s[:, kt : kt + 1],
                        )
                    ps3 = psp3.tile([128, H2], f32, tag="ps3")
                    for kt in range(k2):
                        nc.tensor.matmul(
                            ps3[:],
                            lhsT=h2T[:, kt, :],
                            rhs=wg2_s[:, kt * H2 : (kt + 1) * H2],
                            start=(kt == 0),
                            stop=(kt == k2 - 1),
                        )
                    g2t = tp3.tile([128, H2], bf, tag="g2")
                    nc.vector.tensor_copy(g2t[:], ps3[:])
                    if b < bA:
                        nc.sync.dma_start(
                            g2_localA[b * 128 : b * 128 + 128, :], g2t[:]
                        )
                    else:
                        roff = b * 128 - rA
                        nc.sync.dma_start(
                            g2_localB[roff : roff + 128, :], g2t[:]
                        )

                def post1(t, g):
                    hi_blk = min((g + 1) * G, nb) - 1
                    if t == 1 and not agdone[0] and hi_blk >= bA - 1:
                        nc.gpsimd.collective_compute(
                            "AllGather",
                            ALU.bypass,
                            replica_groups=rg,
                            ins=[g2_localA[:, :]],
                            outs=[t2A[:, :]],
                        )
                        agdone[0] = True

                spmm(tc, s2, nc, (t1A, t1B), idx_s, ident, "a", cb1, post1)

            nc.gpsimd.collective_compute(
                "AllGather",
                ALU.bypass,
                replica_groups=rg,
                ins=[g2_localB[:, :]],
                outs=[t2B[:, :]],
            )

            # ---------------- spmm2 fused with L4
            with ExitStack() as s3:
                psp4 = s3.enter_context(
                    tc.tile_pool(name="ps4", bufs=2, space="PSUM")
                )
                tp4 = s3.enter_context(tc.tile_pool(name="l4t", bufs=3))

                def cb2(b, psT):
                    h3T = tp4.tile([128, k2, 128], bf, tag="h3T")
                    for kt in range(k2):
                        nc.scalar.activation(
                            h3T[:, kt, :],
                            psT[:, kt * 128 : kt * 128 + 128],
                            AF.Relu,
                            bias=bg2_s[:, kt : kt + 1],
                        )
                    ps4 = psp4.tile([128, OUT], f32, tag="ps")
                    for kt in range(k2):
                        nc.tensor.matmul(
                            ps4[:],
                            lhsT=h3T[:, kt, :],
                            rhs=wl2_s[:, kt * OUT : (kt + 1) * OUT],
                            start=(kt == 0),
                            stop=False,
                        )
                    nc.tensor.matmul(
                        ps4[:],
                        lhsT=ones_t[:1, :],
                        rhs=bl2_s[:1, :],
                        start=False,
                        stop=True,
                    )
                    yt = tp4.tile([128, OUT], f32, tag="y")
                    nc.vector.tensor_copy(yt[:], ps4[:])
                    nc.sync.dma_start(y_d[b * 128 : (b + 1) * 128, :], yt[:])

                spmm(tc, s3, nc, (t2A, t2B), idx_s, ident, "b", cb2)

    nc.compile()
    return nc


# ---------------------------------------------------------------- driver

_CACHE = {}


def run(inputs, cfg: Cfg = FULL, trace=False, tmpdir=None):
    meta, in_maps = prep_inputs(cfg, inputs)
    key = (cfg, meta["totch"], meta["idxcols"])
    if key not in _CACHE:
        _CACHE[key] = build(cfg, meta)
    nc = _CACHE[key]
    res = run_bass_kernel_spmd(
        nc,
        in_maps,
        core_ids=list(range(cfg.n_cores)),
        trace=trace,
        tmpdir=tmpdir,
    )
    npc = cfg.nodes_per_core
    out = np.empty((cfg.n_nodes, cfg.out_dim), np.float32)
    for c in range(cfg.n_cores):
        lo = c * npc
        hi = min((c + 1) * npc, cfg.n_nodes)
        out[lo:hi] = res.results[c]["y"][: hi - lo]
    return out, res


def kernel(**inputs) -> np.ndarray:
    out, _ = run(inputs, FULL, trace=False)
    return out


# revision 15
# speedup vs baseline: 1.4749x; 1.0376x over previous
"""GNN message-passing kernel for 8 Trainium2 NeuronCores (Bass/Tile).

v3: host-built one-hot P matrices streamed from HBM, flipped spmm
matmuls (gathered rows stationary, P streamed) producing transposed
psums so relu+bias run natively on the scalar engine and L3a/L4 need
no transposes; node table split A/B with pipelined AllGathers; phased
(A-then-B) spmm with bf16 partial flush; gathers rotate across 4 SWDGE
queues with an enlarged descriptor-ring carveout.
"""

import math
from contextlib import ExitStack
from dataclasses import dataclass

import ml_dtypes
import numpy as np

import concourse.bass as bass
import concourse.mybir as mybir
import concourse.tile as tile
from concourse import bacc
from concourse.bass_utils import run_bass_kernel_spmd
from concourse.masks import make_identity

BF16 = ml_dtypes.bfloat16
AF = mybir.ActivationFunctionType
ALU = mybir.AluOpType


@dataclass(frozen=True)
class Cfg:
    n_nodes: int = 50000
    n_edges: int = 800000
    in_dim: int = 512
    h1: int = 512
    h2: int = 256
    out_dim: int = 128
    n_cores: int = 8
    group_blocks: int = 5  # row-blocks per gather group

    @property
    def nodes_per_core(self):
        return math.ceil(self.n_nodes / self.n_cores)

    @property
    def npad(self):  # per-core padded nodes
        return math.ceil(self.nodes_per_core / 128) * 128

    @property
    def nblocks(self):
        return self.npad // 128

    @property
    def blocksA(self):  # blocks in table A (per-rank row half)
        return (self.nblocks + 1) // 2

    @property
    def rowsA(self):
        return self.blocksA * 128

    @property
    def rowsB(self):
        return self.npad - self.rowsA

    @property
    def ngroups(self):
        return math.ceil(self.nblocks / self.group_blocks)


FULL = Cfg()


# ---------------------------------------------------------------- host prep


def edge_structure(cfg: Cfg, edge_row, edge_col, edge_weight):
    """Per-core edge streams with SPMD-uniform chunk counts.

    Chunk order (global): table t (A=0,B=1) -> group g -> block b -> chunk.
    per_core[c]: idx [128, idxcols] i16; pmat [128, totch*128] bf16 with
    pmat[slot, jc*128 + r] = w for the edge at (chunk jc, slot) with local
    destination row r.
    """
    nc_, npad, nb, G = cfg.n_cores, cfg.npad, cfg.nblocks, cfg.group_blocks
    npc = cfg.nodes_per_core
    bA, rA, rB = cfg.blocksA, cfg.rowsA, cfg.rowsB
    assert max(rA, rB) * nc_ <= 32767, "tables must fit int16 indices"

    core_of = edge_row // npc
    lr_all = edge_row - core_of * npc
    cc = edge_col // npc
    cl = edge_col - cc * npc
    tb_all = (cl >= rA).astype(np.int64)
    tr_all = np.where(tb_all == 1, cc * rB + (cl - rA), cc * rA + cl)
    blk_all = lr_all // 128

    counts = np.zeros((nc_, nb, 2), np.int64)
    np.add.at(counts, (core_of, blk_all, tb_all), 1)
    chunks_bt = np.ceil(counts / 128.0).astype(np.int64).max(axis=0)  # [nb,2]
    chunks_bt = np.maximum(chunks_bt, 1)  # every (block, table) has >=1 chunk

    ngroups = cfg.ngroups
    nch = [[0] * ngroups, [0] * ngroups]
    chunk_blk = [[[] for _ in range(ngroups)] for _ in range(2)]
    off16 = [[0] * ngroups, [0] * ngroups]
    offch = [[0] * ngroups, [0] * ngroups]
    tot16 = 0
    totch = 0
    for t in (0, 1):
        for g in range(ngroups):
            blocks = range(g * G, min((g + 1) * G, nb))
            off16[t][g] = tot16
            offch[t][g] = totch
            n = 0
            for b in blocks:
                cb_ = int(chunks_bt[b, t])
                chunk_blk[t][g].extend([b] * cb_)
                n += cb_
            nch[t][g] = n
            tot16 += n * 8
            totch += n

    meta = dict(
        nch=nch,
        chunk_blk=chunk_blk,
        off16=off16,
        offch=offch,
        totch=totch,
        idxcols=max(tot16, 8),
        chunks_bt=chunks_bt,
    )

    per_core = []
    for c in range(nc_):
        m = core_of == c
        lrc, trc, tbc, wc, blkc = (
            lr_all[m],
            tr_all[m],
            tb_all[m],
            edge_weight[m],
            blk_all[m],
        )
        idx_flat = np.zeros(meta["idxcols"] * 16, np.int16)
        pmat = np.zeros((128, totch * 128), BF16)
        for t in (0, 1):
            for g in range(ngroups):
                blocks = range(g * G, min((g + 1) * G, nb))
                jc = offch[t][g]
                base16 = off16[t][g]
                i_call = 0
                for b in blocks:
                    cb_ = int(chunks_bt[b, t])
                    sel = (blkc == b) & (tbc == t)
                    e_tr = trc[sel]
                    e_lr = lrc[sel]
                    e_w = wc[sel]
                    ne = len(e_tr)
                    vals = np.zeros(cb_ * 128, np.int64)
                    vals[:ne] = e_tr
                    ii = np.arange(cb_ * 128) + i_call
                    idx_flat[(base16 + ii // 16) * 16 + (ii % 16)] = vals.astype(
                        np.int16
                    )
                    i_call += cb_ * 128
                    if ne:
                        i_in = np.arange(ne)
                        jj = jc + i_in // 128
                        slot = i_in % 128
                        r = e_lr - b * 128
                        pmat[slot, jj * 128 + r] = e_w.astype(BF16)
                    jc += cb_
        idx_mat = np.tile(idx_flat.reshape(meta["idxcols"], 16).T, (8, 1))
        per_core.append(
            dict(
                idx=np.ascontiguousarray(idx_mat),
                pmat=pmat,
            )
        )
    return meta, per_core


def prep_inputs(cfg: Cfg, inputs):
    """Shard + lay out all per-core input tensors."""
    f = inputs["features"].astype(np.float32)
    meta, per_edge = edge_structure(
        cfg,
        inputs["edge_row"].astype(np.int64),
        inputs["edge_col"].astype(np.int64),
        inputs["edge_weight"].astype(np.float32),
    )
    kin = cfg.in_dim // 128
    k1 = cfg.h1 // 128
    k2 = cfg.h2 // 128

    def wlayout(w, kt):
        K, M = w.shape
        return (
            w.reshape(kt, 128, M).transpose(1, 0, 2).reshape(128, kt * M)
        ).astype(BF16)

    w1 = wlayout(inputs["W_lin1"].astype(np.float32), kin)
    wg1 = wlayout(inputs["W_g1"].astype(np.float32), k1)
    wg2 = wlayout(inputs["W_g2"].astype(np.float32), k2)
    wl2 = wlayout(inputs["W_lin2"].astype(np.float32), k2)
    b1 = inputs["b_lin1"].astype(np.float32).reshape(kin, 128).T.copy()
    # per-partition bias columns for the transposed spmm outputs
    bg1 = inputs["b_g1"].astype(np.float32).reshape(k2, 128).T.copy()
    bg2 = inputs["b_g2"].astype(np.float32).reshape(k2, 128).T.copy()
    bl2 = inputs["b_lin2"].astype(BF16).reshape(1, cfg.out_dim)

    npc, npad = cfg.nodes_per_core, cfg.npad
    in_maps = []
    for c in range(cfg.n_cores):
        lo = c * npc
        hi = min((c + 1) * npc, cfg.n_nodes)
        xc = np.zeros((npad, cfg.in_dim), np.float32)
        xc[: hi - lo] = f[lo:hi]
        xt = (
            xc.T.reshape(kin, 128, npad)
            .transpose(1, 0, 2)
            .reshape(128, kin * npad)
        ).astype(BF16)
        in_maps.append(
            {
                "xt": np.ascontiguousarray(xt),
                "w1": w1,
                "wg1": wg1,
                "wg2": wg2,
                "wl2": wl2,
                "b1": b1,
                "bg1": bg1,
                "bg2": bg2,
                "bl2": bl2,
                "idx": per_edge[c]["idx"],
                "pmat": per_edge[c]["pmat"],
            }
        )
    return meta, in_maps


# ---------------------------------------------------------------- kernel IR


def build(cfg: Cfg, meta):
    nc = bacc.Bacc(
        "TRN2",
        target_bir_lowering=False,
        debug=False,
        num_devices=cfg.n_cores,
        num_swdge_queues=4,
    )
    bf = mybir.dt.bfloat16
    f32 = mybir.dt.float32
    i16 = mybir.dt.int16
    kin = cfg.in_dim // 128
    k1 = cfg.h1 // 128
    k2 = cfg.h2 // 128
    npad, nb, G, H2, OUT = (
        cfg.npad,
        cfg.nblocks,
        cfg.group_blocks,
        cfg.h2,
        cfg.out_dim,
    )
    bA, rA, rB = cfg.blocksA, cfg.rowsA, cfg.rowsB
    totch = meta["totch"]
    ngroups = cfg.ngroups

    xt_d = nc.dram_tensor("xt", [128, kin * npad], bf, kind="ExternalInput").ap()
    w1_d = nc.dram_tensor("w1", [128, kin * cfg.h1], bf, kind="ExternalInput").ap()
    wg1_d = nc.dram_tensor("wg1", [128, k1 * H2], bf, kind="ExternalInput").ap()
    wg2_d = nc.dram_tensor("wg2", [128, k2 * H2], bf, kind="ExternalInput").ap()
    wl2_d = nc.dram_tensor("wl2", [128, k2 * OUT], bf, kind="ExternalInput").ap()
    b1_d = nc.dram_tensor("b1", [128, kin], f32, kind="ExternalInput").ap()
    bg1_d = nc.dram_tensor("bg1", [128, k2], f32, kind="ExternalInput").ap()
    bg2_d = nc.dram_tensor("bg2", [128, k2], f32, kind="ExternalInput").ap()
    bl2_d = nc.dram_tensor("bl2", [1, OUT], bf, kind="ExternalInput").ap()
    idx_d = nc.dram_tensor(
        "idx", [128, meta["idxcols"]], i16, kind="ExternalInput"
    ).ap()
    pmat_d = nc.dram_tensor(
        "pmat", [128, totch * 128], bf, kind="ExternalInput"
    ).ap()
    y_d = nc.dram_tensor("y", [npad, OUT], f32, kind="ExternalOutput").ap()

    g1_localA = nc.dram_tensor("g1_localA", [rA, H2], bf).ap()
    g1_localB = nc.dram_tensor("g1_localB", [rB, H2], bf).ap()
    g2_localA = nc.dram_tensor("g2_localA", [rA, H2], bf).ap()
    g2_localB = nc.dram_tensor("g2_localB", [rB, H2], bf).ap()
    t1A = nc.dram_tensor("t1A", [rA * cfg.n_cores, H2], bf, addr_space="Shared").ap()
    t1B = nc.dram_tensor("t1B", [rB * cfg.n_cores, H2], bf, addr_space="Shared").ap()
    t2A = nc.dram_tensor("t2A", [rA * cfg.n_cores, H2], bf, addr_space="Shared").ap()
    t2B = nc.dram_tensor("t2B", [rB * cfg.n_cores, H2], bf, addr_space="Shared").ap()

    rg = [list(range(cfg.n_cores))]
    qstate = [0]

    def next_q():
        q = qstate[0]
        qstate[0] = (q + 1) % 4
        return q

    # last global chunk index per (t, block)
    last_chunk = {}
    for t in (0, 1):
        for g in range(ngroups):
            for j, b in enumerate(meta["chunk_blk"][t][g]):
                last_chunk[(t, b)] = meta["offch"][t][g] + j

    def spmm(tc, ctx, nc, tabs, idx_s, ident, tag, out_cb, post_group=None):
        """Phased weighted segment-sum with transposed psums.

        psum half h of block b accumulates sum_e w_e * h[col_e, h*128:...]^T
        -> [feat 128, row 128]."""
        gp = ctx.enter_context(tc.tile_pool(name=f"g{tag}", bufs=4))
        pp = ctx.enter_context(tc.tile_pool(name=f"pm{tag}", bufs=3))
        sp = ctx.enter_context(
            tc.tile_pool(name=f"ps{tag}", bufs=G + 1, space="PSUM")
        )
        pap = ctx.enter_context(tc.tile_pool(name=f"pa{tag}", bufs=1))
        partial = pap.tile([128, nb * H2], bf)

        for t in (0, 1):
            for g in range(ngroups):
                blocks = list(range(g * G, min((g + 1) * G, nb)))
                n = meta["nch"][t][g]
                gt = gp.tile([128, n, H2], bf, tag="g")
                for lo in range(0, n, 15):
                    ns = min(15, n - lo)
                    o16 = meta["off16"][t][g] + lo * 8
                    nc.gpsimd.dma_gather(
                        out_ap=gt[:, lo : lo + ns, :],
                        in_ap=tabs[t][:, :],
                        idxs_ap=idx_s[:, o16 : o16 + ns * 8],
                        num_idxs=ns * 128,
                        num_idxs_reg=ns * 128,
                        elem_size=H2,
                        single_packet=False,
                        queue_num=next_q(),
                    )
                ptile = pp.tile([128, n * 128], bf, tag="p")
                nc.sync.dma_start(
                    ptile[:],
                    pmat_d[
                        :,
                        meta["offch"][t][g] * 128 : (meta["offch"][t][g] + n)
                        * 128,
                    ],
                )
                psums = {
                    b: sp.tile(
                        [128, H2], f32, tag="ps", name=f"ps{tag}_{b}"
                    )
                    for b in blocks
                }
                # single full-width start per block: start_tensor_calc
                # zeroes the whole 2KB psum bank, so per-half starts would
                # wipe each other
                for b in blocks:
                    if t == 0:
                        nc.tensor.matmul(
                            psums[b][:],
                            lhsT=ones_t[:1, :],
                            rhs=zrow[:1, :],
                            start=True,
                            stop=False,
                        )
                    else:
                        nc.tensor.matmul(
                            psums[b][:],
                            lhsT=ident[:, :],
                            rhs=partial[:, b * H2 : (b + 1) * H2],
                            start=True,
                            stop=False,
                        )
                for j, b in enumerate(meta["chunk_blk"][t][g]):
                    jc = meta["offch"][t][g] + j
                    for h in (0, 1):
                        nc.tensor.matmul(
                            psums[b][:, h * 128 : h * 128 + 128],
                            lhsT=gt[:, j, h * 128 : h * 128 + 128],
                            rhs=ptile[:, j * 128 : (j + 1) * 128],
                            start=False,
                            stop=(jc == last_chunk[(t, b)] and h == 1),
                        )
                for b in blocks:
                    if t == 0:
                        nc.vector.tensor_copy(
                            partial[:, b * H2 : (b + 1) * H2], psums[b][:]
                        )
                    else:
                        out_cb(b, psums[b])
                if post_group is not None:
                    post_group(t, g)

    with tile.TileContext(nc) as tc:
        with ExitStack() as top:
            const = top.enter_context(tc.tile_pool(name="const", bufs=1))
            w1_s = const.tile([128, kin * cfg.h1], bf)
            nc.sync.dma_start(w1_s[:], w1_d[:, :])
            wg1_s = const.tile([128, k1 * H2], bf)
            nc.sync.dma_start(wg1_s[:], wg1_d[:, :])
            wg2_s = const.tile([128, k2 * H2], bf)
            nc.sync.dma_start(wg2_s[:], wg2_d[:, :])
            wl2_s = const.tile([128, k2 * OUT], bf)
            nc.sync.dma_start(wl2_s[:], wl2_d[:, :])
            b1_s = const.tile([128, kin], f32)
            nc.sync.dma_start(b1_s[:], b1_d[:, :])
            bg1_s = const.tile([128, k2], f32)
            nc.sync.dma_start(bg1_s[:], bg1_d[:, :])
            bg2_s = const.tile([128, k2], f32)
            nc.sync.dma_start(bg2_s[:], bg2_d[:, :])
            bl2_s = const.tile([1, OUT], bf)
            nc.sync.dma_start(bl2_s[:], bl2_d[:, :])
            idx_s = const.tile([128, meta["idxcols"]], i16)
            nc.sync.dma_start(idx_s[:], idx_d[:, :])
            ident = const.tile([128, 128], bf)
            make_identity(nc, ident[:])
            ones_t = const.tile([1, 128], bf)
            nc.gpsimd.memset(ones_t[:], 1.0)
            zrow = const.tile([1, H2], bf)
            nc.gpsimd.memset(zrow[:], 0.0)

            # ---------------- L1 + L2a, A half then B half, AG after each
            with ExitStack() as s1:
                h1p = s1.enter_context(tc.tile_pool(name="h1t", bufs=1))
                h1t = h1p.tile([128, k1 * npad], bf)
                xp = s1.enter_context(tc.tile_pool(name="xt", bufs=1))
                xt_s = xp.tile([128, kin * npad], bf)
                psp = s1.enter_context(
                    tc.tile_pool(name="ps1", bufs=4, space="PSUM")
                )
                psp2 = s1.enter_context(
                    tc.tile_pool(name="ps2", bufs=4, space="PSUM")
                )
                tp2 = s1.enter_context(tc.tile_pool(name="g1t", bufs=3))

                halves = [
                    (0, rA, range(0, bA), g1_localA, t1A),
                    (rA, npad, range(bA, nb), g1_localB, t1B),
                ]
                for lo_n, hi_n, blocks, gl, tab in halves:
                    for kt in range(kin):
                        nc.sync.dma_start(
                            xt_s[:, kt * npad + lo_n : kt * npad + hi_n],
                            xt_d[:, kt * npad + lo_n : kt * npad + hi_n],
                        )
                    nsl = [
                        (a, min(a + 512, hi_n))
                        for a in range(lo_n, hi_n, 512)
                    ]
                    for f1t in range(k1):
                        for a, b_ in nsl:
                            nw = b_ - a
                            ps = psp.tile([128, 512], f32, tag="ps")
                            for kt in range(kin):
                                nc.tensor.matmul(
                                    ps[:, :nw],
                                    lhsT=w1_s[
                                        :,
                                        kt * cfg.h1
                                        + f1t * 128 : kt * cfg.h1
                                        + f1t * 128
                                        + 128,
                                    ],
                                    rhs=xt_s[:, kt * npad + a : kt * npad + b_],
                                    start=(kt == 0),
                                    stop=(kt == kin - 1),
                                )
                            nc.scalar.activation(
                                h1t[:, f1t * npad + a : f1t * npad + b_],
                                ps[:, :nw],
                                AF.Sigmoid,
                                bias=b1_s[:, f1t : f1t + 1],
                            )
                    for b in blocks:
                        ps = psp2.tile([128, H2], f32, tag="ps")
                        for kt in range(k1):
                            nc.tensor.matmul(
                                ps[:],
                                lhsT=h1t[
                                    :,
                                    kt * npad + b * 128 : kt * npad + b * 128 + 128,
                                ],
                                rhs=wg1_s[:, kt * H2 : (kt + 1) * H2],
                                start=(kt == 0),
                                stop=(kt == k1 - 1),
                            )
                        g1tile = tp2.tile([128, H2], bf, tag="g1")
                        nc.vector.tensor_copy(g1tile[:], ps[:])
                        roff = b * 128 - lo_n
                        nc.sync.dma_start(
                            gl[roff : roff + 128, :], g1tile[:]
                        )
                    nc.gpsimd.collective_compute(
                        "AllGather",
                        ALU.bypass,
                        replica_groups=rg,
                        ins=[gl[:, :]],
                        outs=[tab[:, :]],
                    )

            # ---------------- spmm1 fused with L3a; AG2 halves pipelined
            agdone = [False]

            with ExitStack() as s2:
                psp3 = s2.enter_context(
                    tc.tile_pool(name="ps3", bufs=2, space="PSUM")
                )
                tp3 = s2.enter_context(tc.tile_pool(name="l3t", bufs=3))

                def cb1(b, psT):
                    h2T = tp3.tile([128, k2, 128], bf, tag="h2T")
                    for kt in range(k2):
                        nc.scalar.activation(
                            h2T[:, kt, :],
                            psT[:, kt * 128 : kt * 128 + 128],
                            AF.Relu,
                            bias=bg1_s[:, kt : kt + 1],
                        )
                    ps3 = psp3.tile([128, H2], f32, tag="ps3")
                    for kt in range(k2):
                        nc.tensor.matmul(
                            ps3[:],
                            lhsT=h2T[:, kt, :],
                            rhs=wg2_s[:, kt * H2 : (kt + 1) * H2],
                            start=(kt == 0),
                            stop=(kt == k2 - 1),
                        )
                    g2t = tp3.tile([128, H2], bf, tag="g2")
                    nc.vector.tensor_copy(g2t[:], ps3[:])
                    if b < bA:
                        nc.sync.dma_start(
                            g2_localA[b * 128 : b * 128 + 128, :], g2t[:]
                        )
                    else:
                        roff = b * 128 - rA
                        nc.sync.dma_start(
                            g2_localB[roff : roff + 128, :], g2t[:]
                        )

                def post1(t, g):
                    hi_blk = min((g + 1) * G, nb) - 1
                    if t == 1 and not agdone[0] and hi_blk >= bA - 1:
                        nc.gpsimd.collective_compute(
                            "AllGather",
                            ALU.bypass,
                            replica_groups=rg,
                            ins=[g2_localA[:, :]],
                            outs=[t2A[:, :]],
                        )
                        agdone[0] = True

                spmm(tc, s2, nc, (t1A, t1B), idx_s, ident, "a", cb1, post1)

            nc.gpsimd.collective_compute(
                "AllGather",
                ALU.bypass,
                replica_groups=rg,
                ins=[g2_localB[:, :]],
                outs=[t2B[:, :]],
            )

            # ---------------- spmm2 fused with L4
            with ExitStack() as s3:
                psp4 = s3.enter_context(
                    tc.tile_pool(name="ps4", bufs=2, space="PSUM")
                )
                tp4 = s3.enter_context(tc.tile_pool(name="l4t", bufs=3))

                def cb2(b, psT):
                    h3T = tp4.tile([128, k2, 128], bf, tag="h3T")
                    for kt in range(k2):
                        nc.scalar.activation(
                            h3T[:, kt, :],
                            psT[:, kt * 128 : kt * 128 + 128],
                            AF.Relu,
                            bias=bg2_s[:, kt : kt + 1],
                        )
                    ps4 = psp4.tile([128, OUT], f32, tag="ps")
                    for kt in range(k2):
                        nc.tensor.matmul(
                            ps4[:],
                            lhsT=h3T[:, kt, :],
                            rhs=wl2_s[:, kt * OUT : (kt + 1) * OUT],
                            start=(kt == 0),
                            stop=False,
                        )
                    nc.tensor.matmul(
                        ps4[:],
                        lhsT=ones_t[:1, :],
                        rhs=bl2_s[:1, :],
                        start=False,
                        stop=True,
                    )
                    yt = tp4.tile([128, OUT], f32, tag="y")
                    nc.vector.tensor_copy(yt[:], ps4[:])
                    nc.sync.dma_start(y_d[b * 128 : (b + 1) * 128, :], yt[:])

                spmm(tc, s3, nc, (t2A, t2B), idx_s, ident, "b", cb2)

    nc.compile()
    return nc


# ---------------------------------------------------------------- driver

_CACHE = {}


def run(inputs, cfg: Cfg = FULL, trace=False, tmpdir=None):
    meta, in_maps = prep_inputs(cfg, inputs)
    key = (cfg, meta["totch"], meta["idxcols"])
    if key not in _CACHE:
        _CACHE[key] = build(cfg, meta)
    nc = _CACHE[key]
    res = run_bass_kernel_spmd(
        nc,
        in_maps,
        core_ids=list(range(cfg.n_cores)),
        trace=trace,
        tmpdir=tmpdir,
    )
    npc = cfg.nodes_per_core
    out = np.empty((cfg.n_nodes, cfg.out_dim), np.float32)
    for c in range(cfg.n_cores):
        lo = c * npc
        hi = min((c + 1) * npc, cfg.n_nodes)
        out[lo:hi] = res.results[c]["y"][: hi - lo]
    return out, res


def kernel(**inputs) -> np.ndarray:
    out, _ = run(inputs, FULL, trace=False)
    return out
